# revision 16
# baseline (speedup 1.0000x reference)
"""GCN encoder (kNN softmax message passing, 3 layers) on 8 Trainium2 cores.

Contract: kernel(**inputs) takes FULL numpy inputs (as produced by
setup_inputs()) and returns the FULL (B, N, H) float32 output.

Sharding: data-parallel over batch B=16 -> 2 samples per core on 8 cores.

The program is emitted as an explicit software pipeline over the two
samples so every engine stream stays busy (engine instruction streams
execute in order, so cross-sample overlap must exist in emission order):

  selg(s0,0) agg(s0,L0,c0) selg(s1,0) agg(s1,L0,c0) ... selg(s1,3)
  agg(s1,L0,c3) rest(s0,L0) rest(s1,L0) layer(s0,1) layer(s1,1)
  layer(s0,2)+store layer(s1,2)+store

Per selection group (4 dist tiles): DMA tile, negate (ACT), top-8
values (DVE max8) + positions (DVE max_index), softmax over top-5 using
ACT's accumulator for the sum, GPSIMD local_scatter of 6 (value, index)
pairs (5 weights + the 6th index at weight 0) into a dense fp16 row of
a [128, 4, NP] group tile; then an SBUF-source dma_gather(transpose)
repartitions the group's 512 node rows into a wT chunk tile
(neighbor-on-partition) consumed directly by the agg matmuls.

Layers: aggT = h16^T-contract wT (fp16 PE), deltaT = relu(W @ aggT + b)
(fp16 matmul, f32 out), transpose + identity-matmul residual into PSUM,
layernorm stats via grouped DVE bn_stats on the PSUM residual, fused
scale/bias normalize into fp16 h16 (f32 out buffer on the last layer).
"""

from contextlib import ExitStack

import numpy as np

import concourse.bacc as bacc
import concourse.tile as tile
from concourse import mybir
from concourse.bass_utils import run_bass_kernel_spmd

F32 = mybir.dt.float32
F16 = mybir.dt.float16
U16 = mybir.dt.uint16
I16 = mybir.dt.int16

B, N, H, L, K = 16, 2000, 128, 3, 5
N_CORES = 8
SPC = B // N_CORES          # samples per core
NT = (N + 127) // 128       # 16 node tiles (last has 80 rows)
NP = NT * 128               # 2048 padded nodes
NG = 4                      # node tiles per scatter/gather group
NGRP = NT // NG             # 4 groups per sample
CHUNK = 512                 # gather chunk = nodes per wT chunk tile
LN_EPS = 1e-5
AF = mybir.ActivationFunctionType
OP = mybir.AluOpType


class _SampleState:
    __slots__ = ("h16", "wt", "hout")

    def __init__(self):
        self.h16 = None
        self.wt = []
        self.hout = None


def _build_program(trivial_affine, compile=True):
    nc = bacc.Bacc("TRN2", target_bir_lowering=False, debug=False)

    dist = nc.dram_tensor("dist", [SPC, N, N], F32, kind="ExternalInput").ap()
    emb = nc.dram_tensor("emb", [SPC, N, H], F32, kind="ExternalInput").ap()
    wsT = nc.dram_tensor("wsT", [L, H, H], F16, kind="ExternalInput").ap()
    bsin = nc.dram_tensor("bs", [L, H, 1], F32, kind="ExternalInput").ap()
    ident = nc.dram_tensor("ident", [H, H], F16, kind="ExternalInput").ap()
    identf = nc.dram_tensor("identf", [H, H], F32, kind="ExternalInput").ap()
    gidx = nc.dram_tensor("gidx", [128, CHUNK // 16], I16, kind="ExternalInput").ap()
    out = nc.dram_tensor("out", [SPC, N, H], F32, kind="ExternalOutput").ap()
    if not trivial_affine:
        grep = nc.dram_tensor("grep", [L, H, H], F32, kind="ExternalInput").ap()
        brep = nc.dram_tensor("brep", [L, H, H], F32, kind="ExternalInput").ap()

    with tile.TileContext(nc) as tc, ExitStack() as ctx:
        big = ctx.enter_context(tc.tile_pool(name="big", bufs=1))
        dpool = ctx.enter_context(tc.tile_pool(name="dist", bufs=3))
        sel = ctx.enter_context(tc.tile_pool(name="sel", bufs=4))
        wnp = ctx.enter_context(tc.tile_pool(name="wn", bufs=3))
        wtp = ctx.enter_context(tc.tile_pool(name="wt", bufs=1))
        hp = ctx.enter_context(tc.tile_pool(name="h", bufs=2))
        agp = ctx.enter_context(tc.tile_pool(name="ag", bufs=2))
        ln = ctx.enter_context(tc.tile_pool(name="ln", bufs=4))
        ps_agg = ctx.enter_context(tc.tile_pool(name="ps_agg", bufs=2, space="PSUM"))
        ps_d = ctx.enter_context(tc.tile_pool(name="ps_d", bufs=2, space="PSUM"))
        ps_r = ctx.enter_context(tc.tile_pool(name="ps_r", bufs=1, space="PSUM"))

        # constants
        wsT_sb = big.tile([128, L * H], F16, tag="wsT_sb")
        bs_sb = big.tile([128, L], F32, tag="bs_sb")
        ident_sb = big.tile([128, H], F16, tag="ident_sb")
        identf_sb = big.tile([128, H], F32, tag="identf_sb")
        gidx_sb = big.tile([128, CHUNK // 16], I16, tag="gidx_sb")
        eps_sb = big.tile([128, 1], F32, tag="eps_sb")
        if not trivial_affine:
            grep_sb = big.tile([128, L * H], F32, tag="grep_sb")
            brep_sb = big.tile([128, L * H], F32, tag="brep_sb")
        for l in range(L):
            nc.sync.dma_start(wsT_sb[:, l * H:(l + 1) * H], wsT[l, :, :])
            nc.sync.dma_start(bs_sb[:, l:l + 1], bsin[l, :, :])
            if not trivial_affine:
                nc.sync.dma_start(grep_sb[:, l * H:(l + 1) * H], grep[l, :, :])
                nc.sync.dma_start(brep_sb[:, l * H:(l + 1) * H], brep[l, :, :])
        nc.sync.dma_start(ident_sb[:], ident[:, :])
        nc.sync.dma_start(identf_sb[:], identf[:, :])
        nc.sync.dma_start(gidx_sb[:], gidx[:, :])
        nc.vector.memset(eps_sb[:, :], LN_EPS)

        def emit_emb(s, st):
            st.h16 = hp.tile([128, NT, H], F16, tag="h16")
            nc.vector.memset(st.h16[64:128, NT - 1, :], 0.0)
            # f32 -> fp16 cast during the DMA (SWDGE); 2 calls per sample
            nc.gpsimd.dma_start(
                st.h16[:, 0:NT - 1, :],
                emb[s, 0:(NT - 1) * 128, :].rearrange("(t p) h -> p t h", p=128))
            nc.gpsimd.dma_start(st.h16[0:N - (NT - 1) * 128, NT - 1, :],
                                emb[s, (NT - 1) * 128:N, :])

        def emit_sel_group(s, g, st):
            wn = wnp.tile([128, NG, NP], F16, tag="wn")
            for q in range(NG):
                t = g * NG + q
                r0 = t * 128
                pp = min(128, N - r0)
                dt_ = dpool.tile([128, N], F32, tag="dt")
                nc.sync.dma_start(dt_[:pp, :], dist[s, r0:r0 + pp, :])
                # in-place negate: nd = -d
                nc.scalar.activation(dt_[:pp, :], dt_[:pp, :], AF.Copy,
                                     bias=0.0, scale=-1.0)
                m8 = sel.tile([128, 8], F32, tag="m8")
                nc.vector.max(m8[:pp, :], dt_[:pp, :])
                i16t = sel.tile([128, 8], U16, tag="i16")
                nc.vector.max_index(i16t[:pp, :], m8[:pp, :], dt_[:pp, :])
                # softmax over top-5 (shift-free: values in [-1, 0])
                e5 = sel.tile([128, 5], F32, tag="e5")
                z5 = sel.tile([128, 1], F32, tag="z5")
                nc.scalar.activation(e5[:pp, :], m8[:pp, 0:5], AF.Exp,
                                     accum_out=z5[:pp, :])
                r5 = sel.tile([128, 1], F32, tag="r5")
                nc.vector.reciprocal(r5[:pp, :], z5[:pp, :])
                w6 = sel.tile([128, 6], F16, tag="w6")
                nc.vector.memset(w6[:pp, 5:6], 0.0)
                nc.vector.tensor_scalar(w6[:pp, 0:5], e5[:pp, :], r5[:pp, :],
                                        None, OP.mult)
                # zero the m-padding; scatter never writes it
                nc.vector.memset(wn[:, q, N:NP], 0.0)
                if pp < 128:
                    # zero rows the scatter won't touch (pad nodes)
                    base = pp // 32 * 32
                    nc.vector.memset(wn[base:128, q, :], 0.0)
                # slot 5 carries the 6th-best index with weight 0.0
                nc.gpsimd.local_scatter(
                    wn[0:pp, q, 0:N], w6[:pp, :],
                    i16t[:pp, 0:6].bitcast(I16),
                    channels=(pp + 15) // 16 * 16, num_elems=N, num_idxs=6)

            wtg = wtp.tile([128, NT, CHUNK], F16, tag=f"wt{g}")
            st.wt.append(wtg)
            nc.gpsimd.dma_gather(
                out_ap=wtg[:, :, :],
                in_ap=wn[:, :, :],
                idxs_ap=gidx_sb[:, :],
                num_idxs=CHUNK,
                num_idxs_reg=CHUNK,
                elem_size=NP,
                transpose=True,
                sbuf_tokens_per_rank=128,
                sbuf_free_dim_per_rank=NP * 2,
                queue_num=0,
            )

        def emit_agg_chunk(st, l, ch, aggT):
            ps = ps_agg.tile([128, CHUNK], F32, tag="ps_aggT")
            for c in range(NT):
                nc.tensor.matmul(ps[:, :], lhsT=st.h16[:, c, :],
                                 rhs=st.wt[ch][:, c, :],
                                 start=(c == 0), stop=(c == NT - 1))
            nc.any.tensor_copy(aggT[:, ch * CHUNK:(ch + 1) * CHUNK], ps[:, :])

        def emit_layer_rest(s, l, st, aggT):
            # deltaT = relu(W @ aggT + b); f32 (PE transpose needs
            # out dtype == lhsT dtype and r accumulates in f32)
            deltaT = agp.tile([128, NP], F32, tag="deltaT", bufs=1)
            for ch in range(NGRP):
                sl = slice(ch * CHUNK, (ch + 1) * CHUNK)
                psd = ps_d.tile([128, CHUNK], F32, tag="psd")
                nc.tensor.matmul(psd[:, :], lhsT=wsT_sb[:, l * H:(l + 1) * H],
                                 rhs=aggT[:, sl], start=True, stop=True)
                nc.scalar.activation(deltaT[:, sl], psd[:, :], AF.Relu,
                                     bias=bs_sb[:, l:l + 1], scale=1.0)
            # r = delta^T^T + h  (transpose + identity-matmul residual);
            # PSUM flags grouped per 2KB bank.
            r = ps_r.tile([128, NP], F32, tag="r")
            SPB = 512 // 128
            for g in range(NT // SPB):
                ts_ = list(range(g * SPB, (g + 1) * SPB))
                for j, t in enumerate(ts_):
                    sl = slice(t * 128, (t + 1) * 128)
                    nc.tensor.matmul(r[:, sl], lhsT=deltaT[:, sl],
                                     rhs=identf_sb[:, :], is_transpose=True,
                                     start=(j == 0), stop=False)
                for j, t in enumerate(ts_):
                    sl = slice(t * 128, (t + 1) * 128)
                    nc.tensor.matmul(r[:, sl], lhsT=ident_sb[:, :],
                                     rhs=st.h16[:, t, :], start=False,
                                     stop=(j == len(ts_) - 1))
            # layernorm stats
            sq = agp.tile([128, NP], F32, tag="sq", bufs=1)
            nc.scalar.square(sq[:, :], r[:, :])
            sr = ln.tile([128, NT], F32, tag="sr")
            nc.vector.tensor_reduce(
                sr[:, :], r[:, :].rearrange("p (t h) -> p t h", h=H),
                axis=mybir.AxisListType.X, op=OP.add)
            ssq = ln.tile([128, NT], F32, tag="ssq")
            nc.vector.tensor_reduce(
                ssq[:, :], sq[:, :].rearrange("p (t h) -> p t h", h=H),
                axis=mybir.AxisListType.X, op=OP.add)
            mu = ln.tile([128, NT], F32, tag="mu")
            nc.vector.tensor_scalar_mul(mu[:, :], sr[:, :], 1.0 / H)
            var = ln.tile([128, NT], F32, tag="var")
            # var = E[x^2] - mu^2  (+eps folded into sqrt bias)
            nc.vector.tensor_scalar_mul(var[:, :], ssq[:, :], 1.0 / H)
            musq = ln.tile([128, NT], F32, tag="musq")
            nc.vector.tensor_tensor(musq[:, :], mu[:, :], mu[:, :], OP.mult)
            nc.vector.tensor_tensor(var[:, :], var[:, :], musq[:, :], OP.subtract)
            sd = ln.tile([128, NT], F32, tag="sd")
            nc.scalar.activation(sd[:, :], var[:, :], AF.Sqrt,
                                 bias=eps_sb[:, :], scale=1.0)
            sinv = ln.tile([128, NT], F32, tag="sinv")
            nc.vector.reciprocal(sinv[:, :], sd[:, :])
            negmus = ln.tile([128, NT], F32, tag="negmus")
            nc.vector.tensor_tensor(negmus[:, :], mu[:, :], sinv[:, :], OP.mult)
            nc.vector.tensor_scalar(negmus[:, :], negmus[:, :], -1.0, None,
                                    OP.mult)
            # normalize: h = (r - mu) * sinv  [* gamma + beta]
            last = l == L - 1
            if last:
                st.hout = hp.tile([128, NT, H], F32, tag="hout")
                dst = st.hout
            else:
                dst = st.h16
            for t in range(NT):
                sl = slice(t * 128, (t + 1) * 128)
                if trivial_affine:
                    if t % 4 == 3:
                        nc.vector.tensor_scalar(dst[:, t, :], r[:, sl],
                                                sinv[:, t:t + 1],
                                                negmus[:, t:t + 1],
                                                OP.mult, OP.add)
                    else:
                        nc.scalar.activation(dst[:, t, :], r[:, sl],
                                             AF.Identity,
                                             bias=negmus[:, t:t + 1],
                                             scale=sinv[:, t:t + 1])
                else:
                    tmp = ln.tile([128, H], F32, tag="nrmtmp")
                    nc.scalar.activation(tmp[:, :], r[:, sl], AF.Identity,
                                         bias=negmus[:, t:t + 1],
                                         scale=sinv[:, t:t + 1])
                    nc.vector.tensor_tensor(
                        tmp[:, :], tmp[:, :],
                        grep_sb[:, l * H:(l + 1) * H], OP.mult)
                    nc.vector.tensor_tensor(
                        dst[:, t, :], tmp[:, :],
                        brep_sb[:, l * H:(l + 1) * H], OP.add)

        def emit_layer(s, l, st):
            aggT = agp.tile([128, NP], F16, tag="aggT", bufs=3)
            for ch in range(NGRP):
                emit_agg_chunk(st, l, ch, aggT)
            emit_layer_rest(s, l, st, aggT)

        def emit_store(s, st):
            nc.sync.dma_start(
                out[s, 0:(NT - 1) * 128, :].rearrange("(t p) h -> p t h", p=128),
                st.hout[:, 0:NT - 1, :])
            nc.sync.dma_start(out[s, (NT - 1) * 128:N, :],
                              st.hout[0:N - (NT - 1) * 128, NT - 1, :])

        # ---- software-pipelined emission over the SPC samples ----------
        # s0: sel+L0agg interleaved, L0 rest, L1, then L2 agg chunks
        # interleaved with s1's sel groups (the L2 agg chunk g is the last
        # reader of wt{g}, so s1's gather g may immediately follow), L2
        # rest + store; s1: L0 rest, L1, L2, store.
        s0 = _SampleState()
        s1 = _SampleState()
        emit_emb(0, s0)
        emit_emb(1, s1)
        a00 = agp.tile([128, NP], F16, tag="aggT", name="aggT_s0l0", bufs=3)
        for g in range(NGRP):
            emit_sel_group(0, g, s0)
            emit_agg_chunk(s0, 0, g, a00)
        emit_layer_rest(0, 0, s0, a00)
        emit_layer(0, 1, s0)
        a02 = agp.tile([128, NP], F16, tag="aggT", name="aggT_s0l2", bufs=3)
        a10 = agp.tile([128, NP], F16, tag="aggT", name="aggT_s1l0", bufs=3)
        for g in range(NGRP):
            emit_agg_chunk(s0, 2, g, a02)
            emit_sel_group(1, g, s1)
            emit_agg_chunk(s1, 0, g, a10)
        emit_layer_rest(0, 2, s0, a02)
        emit_store(0, s0)
        emit_layer_rest(1, 0, s1, a10)
        emit_layer(1, 1, s1)
        emit_layer(1, 2, s1)
        emit_store(1, s1)

    if compile:
        nc.compile()
    return nc


def _gather_idx_table():
    """Local index i of the gather lives at (partition i%16 + 16*core,
    free i//16), replicated across the 8 gpsimd cores; value = rank-local
    row (rank = i//128 selects the group tile's NG stripes)."""
    vals = np.arange(CHUNK, dtype=np.int16)
    tab = np.zeros((128, CHUNK // 16), dtype=np.int16)
    for g in range(8):
        tab[g * 16:(g + 1) * 16, :] = vals.reshape(CHUNK // 16, 16).T
    return tab


_CACHE = {}


def kernel(node_emb, dist_matrix, Ws, bs, gammas, betas):
    node_emb = np.ascontiguousarray(np.asarray(node_emb, dtype=np.float32))
    dist_matrix = np.ascontiguousarray(np.asarray(dist_matrix, dtype=np.float32))
    Ws = np.asarray(Ws, dtype=np.float32)
    bs = np.asarray(bs, dtype=np.float32)
    gammas = np.asarray(gammas, dtype=np.float32)
    betas = np.asarray(betas, dtype=np.float32)

    trivial = bool(np.all(gammas == 1.0) and np.all(betas == 0.0))
    key = ("prog", trivial)
    if key not in _CACHE:
        _CACHE[key] = _build_program(trivial)
    nc = _CACHE[key]

    wsT = np.ascontiguousarray(np.transpose(Ws, (0, 2, 1)).astype(np.float16))
    bs3 = np.ascontiguousarray(bs[:, :, None])
    gtab = _gather_idx_table()

    in_maps = []
    for c in range(N_CORES):
        m = {
            "dist": dist_matrix[c * SPC:(c + 1) * SPC],
            "emb": node_emb[c * SPC:(c + 1) * SPC],
            "wsT": wsT,
            "bs": bs3,
            "ident": np.eye(H, dtype=np.float16),
            "identf": np.eye(H, dtype=np.float32),
            "gidx": gtab,
        }
        if not trivial:
            m["grep"] = np.ascontiguousarray(
                np.broadcast_to(gammas[:, None, :], (L, H, H)))
            m["brep"] = np.ascontiguousarray(
                np.broadcast_to(betas[:, None, :], (L, H, H)))
        in_maps.append(m)

    res = run_bass_kernel_spmd(nc, in_maps, list(range(N_CORES)))
    kernel.last_results = res
    out = np.concatenate([res.results[c]["out"] for c in range(N_CORES)], axis=0)
    return out


# revision 19
# speedup vs baseline: 1.0064x; 1.0064x over previous
"""GCN encoder (kNN softmax message passing, 3 layers) on 8 Trainium2 cores.

Contract: kernel(**inputs) takes FULL numpy inputs (as produced by
setup_inputs()) and returns the FULL (B, N, H) float32 output.

Sharding: data-parallel over batch B=16 -> 2 samples per core on 8 cores.

The program is emitted as an explicit software pipeline over the two
samples so every engine stream stays busy (engine instruction streams
execute in order, so cross-sample overlap must exist in emission order):

  selg(s0,0) agg(s0,L0,c0) selg(s1,0) agg(s1,L0,c0) ... selg(s1,3)
  agg(s1,L0,c3) rest(s0,L0) rest(s1,L0) layer(s0,1) layer(s1,1)
  layer(s0,2)+store layer(s1,2)+store

Per selection group (4 dist tiles): DMA tile, negate (ACT), top-8
values (DVE max8) + positions (DVE max_index), softmax over top-5 using
ACT's accumulator for the sum, GPSIMD local_scatter of 6 (value, index)
pairs (5 weights + the 6th index at weight 0) into a dense fp16 row of
a [128, 4, NP] group tile; then an SBUF-source dma_gather(transpose)
repartitions the group's 512 node rows into a wT chunk tile
(neighbor-on-partition) consumed directly by the agg matmuls.

Layers: aggT = h16^T-contract wT (fp16 PE), deltaT = relu(W @ aggT + b)
(fp16 matmul, f32 out), transpose + identity-matmul residual into PSUM,
layernorm stats via grouped DVE bn_stats on the PSUM residual, fused
scale/bias normalize into fp16 h16 (f32 out buffer on the last layer).
"""

from contextlib import ExitStack

import numpy as np

import concourse.bacc as bacc
import concourse.tile as tile
from concourse import mybir
from concourse.bass_utils import run_bass_kernel_spmd

F32 = mybir.dt.float32
F16 = mybir.dt.float16
U16 = mybir.dt.uint16
I16 = mybir.dt.int16

B, N, H, L, K = 16, 2000, 128, 3, 5
N_CORES = 8
SPC = B // N_CORES          # samples per core
NT = (N + 127) // 128       # 16 node tiles (last has 80 rows)
NP = NT * 128               # 2048 padded nodes
NG = 4                      # node tiles per scatter/gather group
NGRP = NT // NG             # 4 groups per sample
CHUNK = 512                 # gather chunk = nodes per wT chunk tile
LN_EPS = 1e-5
AF = mybir.ActivationFunctionType
OP = mybir.AluOpType


class _SampleState:
    __slots__ = ("h16", "wt", "hout")

    def __init__(self):
        self.h16 = None
        self.wt = []
        self.hout = None


def _build_program(trivial_affine, compile=True):
    nc = bacc.Bacc("TRN2", target_bir_lowering=False, debug=False)

    dist = nc.dram_tensor("dist", [SPC, N, N], F32, kind="ExternalInput").ap()
    emb = nc.dram_tensor("emb", [SPC, N, H], F32, kind="ExternalInput").ap()
    wsT = nc.dram_tensor("wsT", [L, H, H], F16, kind="ExternalInput").ap()
    bsin = nc.dram_tensor("bs", [L, H, 1], F32, kind="ExternalInput").ap()
    ident = nc.dram_tensor("ident", [H, H], F16, kind="ExternalInput").ap()
    identf = nc.dram_tensor("identf", [H, H], F32, kind="ExternalInput").ap()
    gidx = nc.dram_tensor("gidx", [128, CHUNK // 16], I16, kind="ExternalInput").ap()
    out = nc.dram_tensor("out", [SPC, N, H], F32, kind="ExternalOutput").ap()
    if not trivial_affine:
        grep = nc.dram_tensor("grep", [L, H, H], F32, kind="ExternalInput").ap()
        brep = nc.dram_tensor("brep", [L, H, H], F32, kind="ExternalInput").ap()

    with tile.TileContext(nc) as tc, ExitStack() as ctx:
        big = ctx.enter_context(tc.tile_pool(name="big", bufs=1))
        dpool = ctx.enter_context(tc.tile_pool(name="dist", bufs=3))
        sel = ctx.enter_context(tc.tile_pool(name="sel", bufs=4))
        wnp = ctx.enter_context(tc.tile_pool(name="wn", bufs=3))
        wtp = ctx.enter_context(tc.tile_pool(name="wt", bufs=1))
        hp = ctx.enter_context(tc.tile_pool(name="h", bufs=2))
        agp = ctx.enter_context(tc.tile_pool(name="ag", bufs=2))
        ln = ctx.enter_context(tc.tile_pool(name="ln", bufs=4))
        ps_agg = ctx.enter_context(tc.tile_pool(name="ps_agg", bufs=2, space="PSUM"))
        ps_d = ctx.enter_context(tc.tile_pool(name="ps_d", bufs=2, space="PSUM"))
        ps_r = ctx.enter_context(tc.tile_pool(name="ps_r", bufs=1, space="PSUM"))

        # constants
        wsT_sb = big.tile([128, L * H], F16, tag="wsT_sb")
        bs_sb = big.tile([128, L], F32, tag="bs_sb")
        ident_sb = big.tile([128, H], F16, tag="ident_sb")
        identf_sb = big.tile([128, H], F32, tag="identf_sb")
        gidx_sb = big.tile([128, CHUNK // 16], I16, tag="gidx_sb")
        eps_sb = big.tile([128, 1], F32, tag="eps_sb")
        if not trivial_affine:
            grep_sb = big.tile([128, L * H], F32, tag="grep_sb")
            brep_sb = big.tile([128, L * H], F32, tag="brep_sb")
        for l in range(L):
            nc.sync.dma_start(wsT_sb[:, l * H:(l + 1) * H], wsT[l, :, :])
            nc.sync.dma_start(bs_sb[:, l:l + 1], bsin[l, :, :])
            if not trivial_affine:
                nc.sync.dma_start(grep_sb[:, l * H:(l + 1) * H], grep[l, :, :])
                nc.sync.dma_start(brep_sb[:, l * H:(l + 1) * H], brep[l, :, :])
        nc.sync.dma_start(ident_sb[:], ident[:, :])
        nc.sync.dma_start(identf_sb[:], identf[:, :])
        nc.sync.dma_start(gidx_sb[:], gidx[:, :])
        nc.vector.memset(eps_sb[:, :], LN_EPS)

        def emit_emb(s, st):
            st.h16 = hp.tile([128, NT, H], F16, tag="h16")
            nc.vector.memset(st.h16[64:128, NT - 1, :], 0.0)
            # f32 -> fp16 cast during the DMA (SWDGE); 2 calls per sample
            nc.gpsimd.dma_start(
                st.h16[:, 0:NT - 1, :],
                emb[s, 0:(NT - 1) * 128, :].rearrange("(t p) h -> p t h", p=128))
            nc.gpsimd.dma_start(st.h16[0:N - (NT - 1) * 128, NT - 1, :],
                                emb[s, (NT - 1) * 128:N, :])

        def emit_sel_group(s, g, st):
            wn = wnp.tile([128, NG, NP], F16, tag="wn")
            for q in range(NG):
                t = g * NG + q
                r0 = t * 128
                pp = min(128, N - r0)
                dt_ = dpool.tile([128, N], F32, tag="dt")
                nc.sync.dma_start(dt_[:pp, :], dist[s, r0:r0 + pp, :])
                # in-place negate: nd = -d
                nc.scalar.activation(dt_[:pp, :], dt_[:pp, :], AF.Copy,
                                     bias=0.0, scale=-1.0)
                m8 = sel.tile([128, 8], F32, tag="m8")
                nc.vector.max(m8[:pp, :], dt_[:pp, :])
                i16t = sel.tile([128, 8], U16, tag="i16")
                nc.vector.max_index(i16t[:pp, :], m8[:pp, :], dt_[:pp, :])
                # softmax over top-5 (shift-free: values in [-1, 0])
                e5 = sel.tile([128, 5], F32, tag="e5")
                z5 = sel.tile([128, 1], F32, tag="z5")
                nc.scalar.activation(e5[:pp, :], m8[:pp, 0:5], AF.Exp,
                                     accum_out=z5[:pp, :])
                r5 = sel.tile([128, 1], F32, tag="r5")
                nc.vector.reciprocal(r5[:pp, :], z5[:pp, :])
                w6 = sel.tile([128, 6], F16, tag="w6")
                nc.vector.memset(w6[:pp, 5:6], 0.0)
                nc.vector.tensor_scalar(w6[:pp, 0:5], e5[:pp, :], r5[:pp, :],
                                        None, OP.mult)
                # zero the m-padding; scatter never writes it
                nc.vector.memset(wn[:, q, N:NP], 0.0)
                if pp < 128:
                    # zero rows the scatter won't touch (pad nodes)
                    base = pp // 32 * 32
                    nc.vector.memset(wn[base:128, q, :], 0.0)
                # slot 5 carries the 6th-best index with weight 0.0
                nc.gpsimd.local_scatter(
                    wn[0:pp, q, 0:N], w6[:pp, :],
                    i16t[:pp, 0:6].bitcast(I16),
                    channels=(pp + 15) // 16 * 16, num_elems=N, num_idxs=6)

            wtg = wtp.tile([128, NT, CHUNK], F16, tag=f"wt{g}")
            st.wt.append(wtg)
            nc.gpsimd.dma_gather(
                out_ap=wtg[:, :, :],
                in_ap=wn[:, :, :],
                idxs_ap=gidx_sb[:, :],
                num_idxs=CHUNK,
                num_idxs_reg=CHUNK,
                elem_size=NP,
                transpose=True,
                sbuf_tokens_per_rank=128,
                sbuf_free_dim_per_rank=NP * 2,
                queue_num=0,
                single_packet=False,
            )

        def emit_agg_chunk(st, l, ch, aggT):
            ps = ps_agg.tile([128, CHUNK], F32, tag="ps_aggT")
            for c in range(NT):
                nc.tensor.matmul(ps[:, :], lhsT=st.h16[:, c, :],
                                 rhs=st.wt[ch][:, c, :],
                                 start=(c == 0), stop=(c == NT - 1))
            nc.any.tensor_copy(aggT[:, ch * CHUNK:(ch + 1) * CHUNK], ps[:, :])

        def emit_layer_rest(s, l, st, aggT):
            # deltaT = relu(W @ aggT + b); f32 (PE transpose needs
            # out dtype == lhsT dtype and r accumulates in f32)
            deltaT = agp.tile([128, NP], F32, tag="deltaT", bufs=2)
            for ch in range(NGRP):
                sl = slice(ch * CHUNK, (ch + 1) * CHUNK)
                psd = ps_d.tile([128, CHUNK], F32, tag="psd")
                nc.tensor.matmul(psd[:, :], lhsT=wsT_sb[:, l * H:(l + 1) * H],
                                 rhs=aggT[:, sl], start=True, stop=True)
                nc.scalar.activation(deltaT[:, sl], psd[:, :], AF.Relu,
                                     bias=bs_sb[:, l:l + 1], scale=1.0)
            # r = delta^T^T + h  (transpose + identity-matmul residual);
            # PSUM flags grouped per 2KB bank.
            r = ps_r.tile([128, NP], F32, tag="r")
            SPB = 512 // 128
            for g in range(NT // SPB):
                ts_ = list(range(g * SPB, (g + 1) * SPB))
                for j, t in enumerate(ts_):
                    sl = slice(t * 128, (t + 1) * 128)
                    nc.tensor.matmul(r[:, sl], lhsT=deltaT[:, sl],
                                     rhs=identf_sb[:, :], is_transpose=True,
                                     start=(j == 0), stop=False)
                for j, t in enumerate(ts_):
                    sl = slice(t * 128, (t + 1) * 128)
                    nc.tensor.matmul(r[:, sl], lhsT=ident_sb[:, :],
                                     rhs=st.h16[:, t, :], start=False,
                                     stop=(j == len(ts_) - 1))
            # layernorm stats (sq in fp16: |x|<~30, rel 5e-4 fine for var)
            sq = agp.tile([128, NP], F16, tag="sq", bufs=1)
            nc.scalar.square(sq[:, :], r[:, :])
            sr = ln.tile([128, NT], F32, tag="sr")
            nc.vector.tensor_reduce(
                sr[:, :], r[:, :].rearrange("p (t h) -> p t h", h=H),
                axis=mybir.AxisListType.X, op=OP.add)
            ssq = ln.tile([128, NT], F32, tag="ssq")
            nc.vector.tensor_reduce(
                ssq[:, :], sq[:, :].rearrange("p (t h) -> p t h", h=H),
                axis=mybir.AxisListType.X, op=OP.add)
            mu = ln.tile([128, NT], F32, tag="mu")
            nc.vector.tensor_scalar_mul(mu[:, :], sr[:, :], 1.0 / H)
            var = ln.tile([128, NT], F32, tag="var")
            # var = E[x^2] - mu^2  (+eps folded into sqrt bias)
            nc.vector.tensor_scalar_mul(var[:, :], ssq[:, :], 1.0 / H)
            musq = ln.tile([128, NT], F32, tag="musq")
            nc.vector.tensor_tensor(musq[:, :], mu[:, :], mu[:, :], OP.mult)
            nc.vector.tensor_tensor(var[:, :], var[:, :], musq[:, :], OP.subtract)
            sd = ln.tile([128, NT], F32, tag="sd")
            nc.scalar.activation(sd[:, :], var[:, :], AF.Sqrt,
                                 bias=eps_sb[:, :], scale=1.0)
            sinv = ln.tile([128, NT], F32, tag="sinv")
            nc.vector.reciprocal(sinv[:, :], sd[:, :])
            negmus = ln.tile([128, NT], F32, tag="negmus")
            nc.vector.tensor_tensor(negmus[:, :], mu[:, :], sinv[:, :], OP.mult)
            nc.vector.tensor_scalar(negmus[:, :], negmus[:, :], -1.0, None,
                                    OP.mult)
            # normalize: h = (r - mu) * sinv  [* gamma + beta]
            last = l == L - 1
            if last:
                st.hout = hp.tile([128, NT, H], F32, tag="hout")
                dst = st.hout
            else:
                dst = st.h16
            for t in range(NT):
                sl = slice(t * 128, (t + 1) * 128)
                if trivial_affine:
                    if t % 4 == 3:
                        nc.vector.tensor_scalar(dst[:, t, :], r[:, sl],
                                                sinv[:, t:t + 1],
                                                negmus[:, t:t + 1],
                                                OP.mult, OP.add)
                    else:
                        nc.scalar.activation(dst[:, t, :], r[:, sl],
                                             AF.Identity,
                                             bias=negmus[:, t:t + 1],
                                             scale=sinv[:, t:t + 1])
                else:
                    tmp = ln.tile([128, H], F32, tag="nrmtmp")
                    nc.scalar.activation(tmp[:, :], r[:, sl], AF.Identity,
                                         bias=negmus[:, t:t + 1],
                                         scale=sinv[:, t:t + 1])
                    nc.vector.tensor_tensor(
                        tmp[:, :], tmp[:, :],
                        grep_sb[:, l * H:(l + 1) * H], OP.mult)
                    nc.vector.tensor_tensor(
                        dst[:, t, :], tmp[:, :],
                        brep_sb[:, l * H:(l + 1) * H], OP.add)

        def emit_layer(s, l, st):
            aggT = agp.tile([128, NP], F16, tag="aggT", bufs=3)
            for ch in range(NGRP):
                emit_agg_chunk(st, l, ch, aggT)
            emit_layer_rest(s, l, st, aggT)

        def emit_store(s, st):
            nc.sync.dma_start(
                out[s, 0:(NT - 1) * 128, :].rearrange("(t p) h -> p t h", p=128),
                st.hout[:, 0:NT - 1, :])
            nc.sync.dma_start(out[s, (NT - 1) * 128:N, :],
                              st.hout[0:N - (NT - 1) * 128, NT - 1, :])

        # ---- software-pipelined emission over the SPC samples ----------
        # s0: sel+L0agg interleaved, L0 rest, L1, then L2 agg chunks
        # interleaved with s1's sel groups (the L2 agg chunk g is the last
        # reader of wt{g}, so s1's gather g may immediately follow), L2
        # rest + store; s1: L0 rest, L1, L2, store.
        s0 = _SampleState()
        s1 = _SampleState()
        emit_emb(0, s0)
        emit_emb(1, s1)
        a00 = agp.tile([128, NP], F16, tag="aggT", name="aggT_s0l0", bufs=3)
        for g in range(NGRP):
            emit_sel_group(0, g, s0)
            emit_agg_chunk(s0, 0, g, a00)
        emit_layer_rest(0, 0, s0, a00)
        emit_layer(0, 1, s0)
        a02 = agp.tile([128, NP], F16, tag="aggT", name="aggT_s0l2", bufs=3)
        a10 = agp.tile([128, NP], F16, tag="aggT", name="aggT_s1l0", bufs=3)
        for g in range(NGRP):
            emit_agg_chunk(s0, 2, g, a02)
            emit_sel_group(1, g, s1)
            emit_agg_chunk(s1, 0, g, a10)
        emit_layer_rest(0, 2, s0, a02)
        emit_store(0, s0)
        emit_layer_rest(1, 0, s1, a10)
        emit_layer(1, 1, s1)
        emit_layer(1, 2, s1)
        emit_store(1, s1)

    if compile:
        nc.compile()
    return nc


def _gather_idx_table():
    """Local index i of the gather lives at (partition i%16 + 16*core,
    free i//16), replicated across the 8 gpsimd cores; value = rank-local
    row (rank = i//128 selects the group tile's NG stripes)."""
    vals = np.arange(CHUNK, dtype=np.int16)
    tab = np.zeros((128, CHUNK // 16), dtype=np.int16)
    for g in range(8):
        tab[g * 16:(g + 1) * 16, :] = vals.reshape(CHUNK // 16, 16).T
    return tab


_CACHE = {}


def kernel(node_emb, dist_matrix, Ws, bs, gammas, betas):
    node_emb = np.ascontiguousarray(np.asarray(node_emb, dtype=np.float32))
    dist_matrix = np.ascontiguousarray(np.asarray(dist_matrix, dtype=np.float32))
    Ws = np.asarray(Ws, dtype=np.float32)
    bs = np.asarray(bs, dtype=np.float32)
    gammas = np.asarray(gammas, dtype=np.float32)
    betas = np.asarray(betas, dtype=np.float32)

    trivial = bool(np.all(gammas == 1.0) and np.all(betas == 0.0))
    key = ("prog", trivial)
    if key not in _CACHE:
        _CACHE[key] = _build_program(trivial)
    nc = _CACHE[key]

    wsT = np.ascontiguousarray(np.transpose(Ws, (0, 2, 1)).astype(np.float16))
    bs3 = np.ascontiguousarray(bs[:, :, None])
    gtab = _gather_idx_table()

    in_maps = []
    for c in range(N_CORES):
        m = {
            "dist": dist_matrix[c * SPC:(c + 1) * SPC],
            "emb": node_emb[c * SPC:(c + 1) * SPC],
            "wsT": wsT,
            "bs": bs3,
            "ident": np.eye(H, dtype=np.float16),
            "identf": np.eye(H, dtype=np.float32),
            "gidx": gtab,
        }
        if not trivial:
            m["grep"] = np.ascontiguousarray(
                np.broadcast_to(gammas[:, None, :], (L, H, H)))
            m["brep"] = np.ascontiguousarray(
                np.broadcast_to(betas[:, None, :], (L, H, H)))
        in_maps.append(m)

    res = run_bass_kernel_spmd(nc, in_maps, list(range(N_CORES)))
    kernel.last_results = res
    out = np.concatenate([res.results[c]["out"] for c in range(N_CORES)], axis=0)
    return out


# revision 24
# speedup vs baseline: 1.0298x; 1.0233x over previous
"""GCN encoder (kNN softmax message passing, 3 layers) on 8 Trainium2 cores.

Contract: kernel(**inputs) takes FULL numpy inputs (as produced by
setup_inputs()) and returns the FULL (B, N, H) float32 output.

Sharding: data-parallel over batch B=16 -> 2 samples per core on 8 cores.

The program is emitted as an explicit software pipeline over the two
samples so every engine stream stays busy (engine instruction streams
execute in order, so cross-sample overlap must exist in emission order):

  selg(s0,0) agg(s0,L0,c0) selg(s1,0) agg(s1,L0,c0) ... selg(s1,3)
  agg(s1,L0,c3) rest(s0,L0) rest(s1,L0) layer(s0,1) layer(s1,1)
  layer(s0,2)+store layer(s1,2)+store

Per selection group (4 dist tiles): DMA tile, negate (ACT), top-8
values (DVE max8) + positions (DVE max_index), softmax over top-5 using
ACT's accumulator for the sum, GPSIMD local_scatter of 6 (value, index)
pairs (5 weights + the 6th index at weight 0) into a dense fp16 row of
a [128, 4, NP] group tile; then an SBUF-source dma_gather(transpose)
repartitions the group's 512 node rows into a wT chunk tile
(neighbor-on-partition) consumed directly by the agg matmuls.

Layers: aggT = h16^T-contract wT (fp16 PE), deltaT = relu(W @ aggT + b)
(fp16 matmul, f32 out), transpose + identity-matmul residual into PSUM,
layernorm stats via grouped DVE bn_stats on the PSUM residual, fused
scale/bias normalize into fp16 h16 (f32 out buffer on the last layer).
"""

from contextlib import ExitStack

import numpy as np

import concourse.bacc as bacc
import concourse.tile as tile
from concourse import mybir
from concourse.bass_utils import run_bass_kernel_spmd

F32 = mybir.dt.float32
F16 = mybir.dt.float16
U16 = mybir.dt.uint16
I16 = mybir.dt.int16

B, N, H, L, K = 16, 2000, 128, 3, 5
N_CORES = 8
SPC = B // N_CORES          # samples per core
NT = (N + 127) // 128       # 16 node tiles (last has 80 rows)
NP = NT * 128               # 2048 padded nodes
NG = 4                      # node tiles per scatter/gather group
NGRP = NT // NG             # 4 groups per sample
CHUNK = 512                 # gather chunk = nodes per wT chunk tile
LN_EPS = 1e-5
AF = mybir.ActivationFunctionType
OP = mybir.AluOpType


class _SampleState:
    __slots__ = ("h16", "wt", "hout")

    def __init__(self):
        self.h16 = None
        self.wt = []
        self.hout = None


def _build_program(trivial_affine, compile=True):
    nc = bacc.Bacc("TRN2", target_bir_lowering=False, debug=False)

    dist = nc.dram_tensor("dist", [SPC, N, N], F32, kind="ExternalInput").ap()
    emb = nc.dram_tensor("emb", [SPC, N, H], F32, kind="ExternalInput").ap()
    wsT = nc.dram_tensor("wsT", [L, H, H], F16, kind="ExternalInput").ap()
    bsin = nc.dram_tensor("bs", [L, H, 1], F32, kind="ExternalInput").ap()
    ident = nc.dram_tensor("ident", [H, H], F16, kind="ExternalInput").ap()
    identf = nc.dram_tensor("identf", [H, H], F32, kind="ExternalInput").ap()
    gidx = nc.dram_tensor("gidx", [128, CHUNK // 16], I16, kind="ExternalInput").ap()
    out = nc.dram_tensor("out", [SPC, N, H], F32, kind="ExternalOutput").ap()
    if not trivial_affine:
        grep = nc.dram_tensor("grep", [L, H, H], F32, kind="ExternalInput").ap()
        brep = nc.dram_tensor("brep", [L, H, H], F32, kind="ExternalInput").ap()

    with tile.TileContext(nc) as tc, ExitStack() as ctx:
        big = ctx.enter_context(tc.tile_pool(name="big", bufs=1))
        dpool = ctx.enter_context(tc.tile_pool(name="dist", bufs=5))
        sel = ctx.enter_context(tc.tile_pool(name="sel", bufs=4))
        wnp = ctx.enter_context(tc.tile_pool(name="wn", bufs=3))
        wtp = ctx.enter_context(tc.tile_pool(name="wt", bufs=1))
        hp = ctx.enter_context(tc.tile_pool(name="h", bufs=2))
        hop = ctx.enter_context(tc.tile_pool(name="ho", bufs=1))
        agp = ctx.enter_context(tc.tile_pool(name="ag", bufs=2))
        ln = ctx.enter_context(tc.tile_pool(name="ln", bufs=4))
        ps_agg = ctx.enter_context(tc.tile_pool(name="ps_agg", bufs=2, space="PSUM"))
        ps_d = ctx.enter_context(tc.tile_pool(name="ps_d", bufs=2, space="PSUM"))
        ps_r = ctx.enter_context(tc.tile_pool(name="ps_r", bufs=1, space="PSUM"))

        # constants
        wsT_sb = big.tile([128, L * H], F16, tag="wsT_sb")
        bs_sb = big.tile([128, L], F32, tag="bs_sb")
        ident_sb = big.tile([128, H], F16, tag="ident_sb")
        identf_sb = big.tile([128, H], F32, tag="identf_sb")
        gidx_sb = big.tile([128, CHUNK // 16], I16, tag="gidx_sb")
        eps_sb = big.tile([128, 1], F32, tag="eps_sb")
        if not trivial_affine:
            grep_sb = big.tile([128, L * H], F32, tag="grep_sb")
            brep_sb = big.tile([128, L * H], F32, tag="brep_sb")
        for l in range(L):
            nc.sync.dma_start(wsT_sb[:, l * H:(l + 1) * H], wsT[l, :, :])
            nc.sync.dma_start(bs_sb[:, l:l + 1], bsin[l, :, :])
            if not trivial_affine:
                nc.sync.dma_start(grep_sb[:, l * H:(l + 1) * H], grep[l, :, :])
                nc.sync.dma_start(brep_sb[:, l * H:(l + 1) * H], brep[l, :, :])
        nc.sync.dma_start(ident_sb[:], ident[:, :])
        nc.sync.dma_start(identf_sb[:], identf[:, :])
        nc.sync.dma_start(gidx_sb[:], gidx[:, :])
        nc.vector.memset(eps_sb[:, :], LN_EPS)

        def emit_emb(s, st):
            st.h16 = hp.tile([128, NT, H], F16, tag="h16")
            nc.vector.memset(st.h16[64:128, NT - 1, :], 0.0)
            # f32 -> fp16 cast during the DMA (SWDGE); 2 calls per sample
            nc.gpsimd.dma_start(
                st.h16[:, 0:NT - 1, :],
                emb[s, 0:(NT - 1) * 128, :].rearrange("(t p) h -> p t h", p=128))
            nc.gpsimd.dma_start(st.h16[0:N - (NT - 1) * 128, NT - 1, :],
                                emb[s, (NT - 1) * 128:N, :])

        def emit_sel_group(s, g, st):
            wn = wnp.tile([128, NG, NP], F16, tag="wn")
            for q in range(NG):
                t = g * NG + q
                r0 = t * 128
                pp = min(128, N - r0)
                dt_ = dpool.tile([128, N], F32, tag="dt")
                nc.sync.dma_start(dt_[:pp, :], dist[s, r0:r0 + pp, :])
                # in-place negate: nd = -d
                nc.scalar.activation(dt_[:pp, :], dt_[:pp, :], AF.Copy,
                                     bias=0.0, scale=-1.0)
                m8 = sel.tile([128, 8], F32, tag="m8")
                nc.vector.max(m8[:pp, :], dt_[:pp, :])
                i16t = sel.tile([128, 8], U16, tag="i16")
                nc.vector.max_index(i16t[:pp, :], m8[:pp, :], dt_[:pp, :])
                # softmax over top-5 (shift-free: values in [-1, 0])
                e5 = sel.tile([128, 5], F32, tag="e5")
                z5 = sel.tile([128, 1], F32, tag="z5")
                nc.scalar.activation(e5[:pp, :], m8[:pp, 0:5], AF.Exp,
                                     accum_out=z5[:pp, :])
                r5 = sel.tile([128, 1], F32, tag="r5")
                nc.vector.reciprocal(r5[:pp, :], z5[:pp, :])
                w6 = sel.tile([128, 6], F16, tag="w6")
                nc.vector.memset(w6[:pp, 5:6], 0.0)
                nc.vector.tensor_scalar(w6[:pp, 0:5], e5[:pp, :], r5[:pp, :],
                                        None, OP.mult)
                # zero the m-padding; scatter never writes it
                nc.vector.memset(wn[:, q, N:NP], 0.0)
                if pp < 128:
                    # zero rows the scatter won't touch (pad nodes)
                    base = pp // 32 * 32
                    nc.vector.memset(wn[base:128, q, :], 0.0)
                # slot 5 carries the 6th-best index with weight 0.0
                nc.gpsimd.local_scatter(
                    wn[0:pp, q, 0:N], w6[:pp, :],
                    i16t[:pp, 0:6].bitcast(I16),
                    channels=(pp + 15) // 16 * 16, num_elems=N, num_idxs=6)

            wtg = wtp.tile([128, NT, CHUNK], F16, tag=f"wt{g}")
            st.wt.append(wtg)
            nc.gpsimd.dma_gather(
                out_ap=wtg[:, :, :],
                in_ap=wn[:, :, :],
                idxs_ap=gidx_sb[:, :],
                num_idxs=CHUNK,
                num_idxs_reg=CHUNK,
                elem_size=NP,
                transpose=True,
                sbuf_tokens_per_rank=128,
                sbuf_free_dim_per_rank=NP * 2,
                queue_num=0,
                single_packet=False,
            )

        def emit_agg_chunk(st, l, ch, aggT):
            ps = ps_agg.tile([128, CHUNK], F32, tag="ps_aggT")
            for c in range(NT):
                nc.tensor.matmul(ps[:, :], lhsT=st.h16[:, c, :],
                                 rhs=st.wt[ch][:, c, :],
                                 start=(c == 0), stop=(c == NT - 1))
            nc.any.tensor_copy(aggT[:, ch * CHUNK:(ch + 1) * CHUNK], ps[:, :])

        def emit_layer_rest(s, l, st, aggT):
            # deltaT = relu(W @ aggT + b); f32 (PE transpose needs
            # out dtype == lhsT dtype and r accumulates in f32)
            deltaT = agp.tile([128, NP], F32, tag="deltaT", bufs=2)
            for ch in range(NGRP):
                sl = slice(ch * CHUNK, (ch + 1) * CHUNK)
                psd = ps_d.tile([128, CHUNK], F32, tag="psd")
                nc.tensor.matmul(psd[:, :], lhsT=wsT_sb[:, l * H:(l + 1) * H],
                                 rhs=aggT[:, sl], start=True, stop=True)
                nc.scalar.activation(deltaT[:, sl], psd[:, :], AF.Relu,
                                     bias=bs_sb[:, l:l + 1], scale=1.0)
            # r = delta^T^T + h  (transpose + identity-matmul residual);
            # PSUM flags grouped per 2KB bank.
            r = ps_r.tile([128, NP], F32, tag="r")
            SPB = 512 // 128
            for g in range(NT // SPB):
                ts_ = list(range(g * SPB, (g + 1) * SPB))
                for j, t in enumerate(ts_):
                    sl = slice(t * 128, (t + 1) * 128)
                    nc.tensor.matmul(r[:, sl], lhsT=deltaT[:, sl],
                                     rhs=identf_sb[:, :], is_transpose=True,
                                     start=(j == 0), stop=False)
                for j, t in enumerate(ts_):
                    sl = slice(t * 128, (t + 1) * 128)
                    nc.tensor.matmul(r[:, sl], lhsT=ident_sb[:, :],
                                     rhs=st.h16[:, t, :], start=False,
                                     stop=(j == len(ts_) - 1))
            # layernorm stats (sq in fp16: |x|<~30, rel 5e-4 fine for var)
            sq = agp.tile([128, NP], F16, tag="sq", bufs=1)
            nc.scalar.square(sq[:, :], r[:, :])
            sr = ln.tile([128, NT], F32, tag="sr")
            nc.vector.tensor_reduce(
                sr[:, :], r[:, :].rearrange("p (t h) -> p t h", h=H),
                axis=mybir.AxisListType.X, op=OP.add)
            ssq = ln.tile([128, NT], F32, tag="ssq")
            nc.vector.tensor_reduce(
                ssq[:, :], sq[:, :].rearrange("p (t h) -> p t h", h=H),
                axis=mybir.AxisListType.X, op=OP.add)
            mu = ln.tile([128, NT], F32, tag="mu")
            nc.vector.tensor_scalar_mul(mu[:, :], sr[:, :], 1.0 / H)
            var = ln.tile([128, NT], F32, tag="var")
            # var = E[x^2] - mu^2  (+eps folded into sqrt bias)
            nc.vector.tensor_scalar_mul(var[:, :], ssq[:, :], 1.0 / H)
            musq = ln.tile([128, NT], F32, tag="musq")
            nc.vector.tensor_tensor(musq[:, :], mu[:, :], mu[:, :], OP.mult)
            nc.vector.tensor_tensor(var[:, :], var[:, :], musq[:, :], OP.subtract)
            # sinv = (var+eps)^-0.5 via exp(-0.5*ln(var+eps)): keeps ACT on
            # one table set (exp/ln) -- no sqrt-set reload on the LN path.
            lnv = ln.tile([128, NT], F32, tag="lnv")
            nc.scalar.activation(lnv[:, :], var[:, :], AF.Ln,
                                 bias=eps_sb[:, :], scale=1.0)
            sinv = ln.tile([128, NT], F32, tag="sinv")
            nc.scalar.activation(sinv[:, :], lnv[:, :], AF.Exp,
                                 bias=0.0, scale=-0.5)
            negmus = ln.tile([128, NT], F32, tag="negmus")
            nc.vector.tensor_tensor(negmus[:, :], mu[:, :], sinv[:, :], OP.mult)
            nc.vector.tensor_scalar(negmus[:, :], negmus[:, :], -1.0, None,
                                    OP.mult)
            # normalize: h = (r - mu) * sinv  [* gamma + beta]
            last = l == L - 1
            if last:
                st.hout = hop.tile([128, NT, H], F32, tag="hout")
                dst = st.hout
            else:
                dst = st.h16
            for t in range(NT):
                sl = slice(t * 128, (t + 1) * 128)
                if trivial_affine:
                    if t % 4 == 3:
                        nc.vector.tensor_scalar(dst[:, t, :], r[:, sl],
                                                sinv[:, t:t + 1],
                                                negmus[:, t:t + 1],
                                                OP.mult, OP.add)
                    else:
                        nc.scalar.activation(dst[:, t, :], r[:, sl],
                                             AF.Identity,
                                             bias=negmus[:, t:t + 1],
                                             scale=sinv[:, t:t + 1])
                else:
                    tmp = ln.tile([128, H], F32, tag="nrmtmp")
                    nc.scalar.activation(tmp[:, :], r[:, sl], AF.Identity,
                                         bias=negmus[:, t:t + 1],
                                         scale=sinv[:, t:t + 1])
                    nc.vector.tensor_tensor(
                        tmp[:, :], tmp[:, :],
                        grep_sb[:, l * H:(l + 1) * H], OP.mult)
                    nc.vector.tensor_tensor(
                        dst[:, t, :], tmp[:, :],
                        brep_sb[:, l * H:(l + 1) * H], OP.add)

        def emit_layer(s, l, st):
            aggT = agp.tile([128, NP], F16, tag="aggT", bufs=3)
            for ch in range(NGRP):
                emit_agg_chunk(st, l, ch, aggT)
            emit_layer_rest(s, l, st, aggT)

        def emit_store(s, st):
            nc.sync.dma_start(
                out[s, 0:(NT - 1) * 128, :].rearrange("(t p) h -> p t h", p=128),
                st.hout[:, 0:NT - 1, :])
            nc.sync.dma_start(out[s, (NT - 1) * 128:N, :],
                              st.hout[0:N - (NT - 1) * 128, NT - 1, :])

        # ---- software-pipelined emission over the SPC samples ----------
        # s0: sel+L0agg interleaved, L0 rest, L1, then L2 agg chunks
        # interleaved with s1's sel groups (the L2 agg chunk g is the last
        # reader of wt{g}, so s1's gather g may immediately follow), L2
        # rest + store; s1: L0 rest, L1, L2, store.
        s0 = _SampleState()
        s1 = _SampleState()
        emit_emb(0, s0)
        emit_emb(1, s1)
        a00 = agp.tile([128, NP], F16, tag="aggT", name="aggT_s0l0", bufs=3)
        for g in range(NGRP):
            emit_sel_group(0, g, s0)
            emit_agg_chunk(s0, 0, g, a00)
        emit_layer_rest(0, 0, s0, a00)
        emit_layer(0, 1, s0)
        a02 = agp.tile([128, NP], F16, tag="aggT", name="aggT_s0l2", bufs=3)
        a10 = agp.tile([128, NP], F16, tag="aggT", name="aggT_s1l0", bufs=3)
        for g in range(NGRP):
            emit_agg_chunk(s0, 2, g, a02)
            emit_sel_group(1, g, s1)
            emit_agg_chunk(s1, 0, g, a10)
        emit_layer_rest(0, 2, s0, a02)
        emit_store(0, s0)
        emit_layer_rest(1, 0, s1, a10)
        emit_layer(1, 1, s1)
        emit_layer(1, 2, s1)
        emit_store(1, s1)

    if compile:
        nc.compile()
    return nc


def _gather_idx_table():
    """Local index i of the gather lives at (partition i%16 + 16*core,
    free i//16), replicated across the 8 gpsimd cores; value = rank-local
    row (rank = i//128 selects the group tile's NG stripes)."""
    vals = np.arange(CHUNK, dtype=np.int16)
    tab = np.zeros((128, CHUNK // 16), dtype=np.int16)
    for g in range(8):
        tab[g * 16:(g + 1) * 16, :] = vals.reshape(CHUNK // 16, 16).T
    return tab


_CACHE = {}


def kernel(node_emb, dist_matrix, Ws, bs, gammas, betas):
    node_emb = np.ascontiguousarray(np.asarray(node_emb, dtype=np.float32))
    dist_matrix = np.ascontiguousarray(np.asarray(dist_matrix, dtype=np.float32))
    Ws = np.asarray(Ws, dtype=np.float32)
    bs = np.asarray(bs, dtype=np.float32)
    gammas = np.asarray(gammas, dtype=np.float32)
    betas = np.asarray(betas, dtype=np.float32)

    trivial = bool(np.all(gammas == 1.0) and np.all(betas == 0.0))
    key = ("prog", trivial)
    if key not in _CACHE:
        _CACHE[key] = _build_program(trivial)
    nc = _CACHE[key]

    wsT = np.ascontiguousarray(np.transpose(Ws, (0, 2, 1)).astype(np.float16))
    bs3 = np.ascontiguousarray(bs[:, :, None])
    gtab = _gather_idx_table()

    in_maps = []
    for c in range(N_CORES):
        m = {
            "dist": dist_matrix[c * SPC:(c + 1) * SPC],
            "emb": node_emb[c * SPC:(c + 1) * SPC],
            "wsT": wsT,
            "bs": bs3,
            "ident": np.eye(H, dtype=np.float16),
            "identf": np.eye(H, dtype=np.float32),
            "gidx": gtab,
        }
        if not trivial:
            m["grep"] = np.ascontiguousarray(
                np.broadcast_to(gammas[:, None, :], (L, H, H)))
            m["brep"] = np.ascontiguousarray(
                np.broadcast_to(betas[:, None, :], (L, H, H)))
        in_maps.append(m)

    res = run_bass_kernel_spmd(nc, in_maps, list(range(N_CORES)))
    kernel.last_results = res
    out = np.concatenate([res.results[c]["out"] for c in range(N_CORES)], axis=0)
    return out


# revision 29
# speedup vs baseline: 1.1436x; 1.1104x over previous
"""GCN encoder (kNN softmax message passing, 3 layers) on 8 Trainium2 cores.

Contract: kernel(**inputs) takes FULL numpy inputs (as produced by
setup_inputs()) and returns the FULL (B, N, H) float32 output.

Sharding: data-parallel over batch B=16 -> 2 samples per core on 8 cores.

The program is emitted as an explicit software pipeline over the two
samples so every engine stream stays busy (engine instruction streams
execute in order, so cross-sample overlap must exist in emission order):

  selg(s0,0) agg(s0,L0,c0) selg(s1,0) agg(s1,L0,c0) ... selg(s1,3)
  agg(s1,L0,c3) rest(s0,L0) rest(s1,L0) layer(s0,1) layer(s1,1)
  layer(s0,2)+store layer(s1,2)+store

Per selection group (4 dist tiles): DMA tile, negate (ACT), top-8
values (DVE max8) + positions (DVE max_index), softmax over top-5 using
ACT's accumulator for the sum, GPSIMD local_scatter of 6 (value, index)
pairs (5 weights + the 6th index at weight 0) into a dense fp16 row of
a [128, 4, NP] group tile; then an SBUF-source dma_gather(transpose)
repartitions the group's 512 node rows into a wT chunk tile
(neighbor-on-partition) consumed directly by the agg matmuls.

Layers: aggT = h16^T-contract wT (fp16 PE), deltaT = relu(W @ aggT + b)
(fp16 matmul, f32 out), transpose + identity-matmul residual into PSUM,
layernorm stats via grouped DVE bn_stats on the PSUM residual, fused
scale/bias normalize into fp16 h16 (f32 out buffer on the last layer).
"""

from contextlib import ExitStack

import numpy as np

import concourse.bacc as bacc
import concourse.tile as tile
from concourse import mybir
from concourse.bass_utils import run_bass_kernel_spmd

F32 = mybir.dt.float32
F16 = mybir.dt.float16
U16 = mybir.dt.uint16
I16 = mybir.dt.int16

B, N, H, L, K = 16, 2000, 128, 3, 5
N_CORES = 8
SPC = B // N_CORES          # samples per core
NT = (N + 127) // 128       # 16 node tiles (last has 80 rows)
NP = NT * 128               # 2048 padded nodes
NG = 4                      # node tiles per scatter/gather group
NGRP = NT // NG             # 4 groups per sample
CHUNK = 512                 # gather chunk = nodes per wT chunk tile
LN_EPS = 1e-5
AF = mybir.ActivationFunctionType
OP = mybir.AluOpType


class _SampleState:
    __slots__ = ("h16", "wt", "hout")

    def __init__(self):
        self.h16 = None
        self.wt = []
        self.hout = None


def _build_program(trivial_affine, compile=True):
    nc = bacc.Bacc("TRN2", target_bir_lowering=False, debug=False)

    dist = nc.dram_tensor("dist", [SPC, N, N], F32, kind="ExternalInput").ap()
    emb = nc.dram_tensor("emb", [SPC, N, H], F32, kind="ExternalInput").ap()
    wsT = nc.dram_tensor("wsT", [L, H, H], F16, kind="ExternalInput").ap()
    bsin = nc.dram_tensor("bs", [L, H, 1], F32, kind="ExternalInput").ap()
    ident = nc.dram_tensor("ident", [H, H], F16, kind="ExternalInput").ap()
    identf = nc.dram_tensor("identf", [H, H], F32, kind="ExternalInput").ap()
    gidx = nc.dram_tensor("gidx", [128, CHUNK // 16], I16, kind="ExternalInput").ap()
    out = nc.dram_tensor("out", [SPC, N, H], F32, kind="ExternalOutput").ap()
    if not trivial_affine:
        grep = nc.dram_tensor("grep", [L, H, H], F32, kind="ExternalInput").ap()
        brep = nc.dram_tensor("brep", [L, H, H], F32, kind="ExternalInput").ap()

    with tile.TileContext(nc) as tc, ExitStack() as ctx:
        big = ctx.enter_context(tc.tile_pool(name="big", bufs=1))
        dpool = ctx.enter_context(tc.tile_pool(name="dist", bufs=4))
        sel = ctx.enter_context(tc.tile_pool(name="sel", bufs=4))
        wnp = ctx.enter_context(tc.tile_pool(name="wn", bufs=4))
        wtp = ctx.enter_context(tc.tile_pool(name="wt", bufs=1))
        hp = ctx.enter_context(tc.tile_pool(name="h", bufs=2))
        hop = ctx.enter_context(tc.tile_pool(name="ho", bufs=1))
        agp = ctx.enter_context(tc.tile_pool(name="ag", bufs=2))
        ln = ctx.enter_context(tc.tile_pool(name="ln", bufs=4))
        ps_agg = ctx.enter_context(tc.tile_pool(name="ps_agg", bufs=2, space="PSUM"))
        ps_d = ctx.enter_context(tc.tile_pool(name="ps_d", bufs=2, space="PSUM"))
        ps_r = ctx.enter_context(tc.tile_pool(name="ps_r", bufs=1, space="PSUM"))

        # constants
        wsT_sb = big.tile([128, L * H], F16, tag="wsT_sb")
        bs_sb = big.tile([128, L], F32, tag="bs_sb")
        ident_sb = big.tile([128, H], F16, tag="ident_sb")
        identf_sb = big.tile([128, H], F32, tag="identf_sb")
        gidx_sb = big.tile([128, CHUNK // 16], I16, tag="gidx_sb")
        eps_sb = big.tile([128, 1], F32, tag="eps_sb")
        if not trivial_affine:
            grep_sb = big.tile([128, L * H], F32, tag="grep_sb")
            brep_sb = big.tile([128, L * H], F32, tag="brep_sb")
        for l in range(L):
            nc.sync.dma_start(wsT_sb[:, l * H:(l + 1) * H], wsT[l, :, :])
            nc.sync.dma_start(bs_sb[:, l:l + 1], bsin[l, :, :])
            if not trivial_affine:
                nc.sync.dma_start(grep_sb[:, l * H:(l + 1) * H], grep[l, :, :])
                nc.sync.dma_start(brep_sb[:, l * H:(l + 1) * H], brep[l, :, :])
        nc.sync.dma_start(ident_sb[:], ident[:, :])
        nc.sync.dma_start(identf_sb[:], identf[:, :])
        nc.sync.dma_start(gidx_sb[:], gidx[:, :])
        nc.vector.memset(eps_sb[:, :], LN_EPS)

        def emit_emb(s, st):
            st.h16 = hp.tile([128, NT, H], F16, tag="h16")
            nc.vector.memset(st.h16[64:128, NT - 1, :], 0.0)
            # f32 -> fp16 cast during the DMA (SWDGE); 2 calls per sample
            nc.gpsimd.dma_start(
                st.h16[:, 0:NT - 1, :],
                emb[s, 0:(NT - 1) * 128, :].rearrange("(t p) h -> p t h", p=128))
            nc.gpsimd.dma_start(st.h16[0:N - (NT - 1) * 128, NT - 1, :],
                                emb[s, (NT - 1) * 128:N, :])

        def emit_gather(st, g, wn):
            wtg = wtp.tile([128, NT, CHUNK], F16, tag=f"wt{g}",
                           name=f"wt{g}_{len(st.wt)}")
            st.wt.append(wtg)
            nc.gpsimd.dma_gather(
                out_ap=wtg[:, :, :],
                in_ap=wn[:, :, :],
                idxs_ap=gidx_sb[:, :],
                num_idxs=CHUNK,
                num_idxs_reg=CHUNK,
                elem_size=NP,
                transpose=True,
                sbuf_tokens_per_rank=128,
                sbuf_free_dim_per_rank=NP * 2,
                queue_num=0,
                single_packet=False,
            )

        def emit_sel_group(s, g, st, gather=True):
            wn = wnp.tile([128, NG, NP], F16, tag="wn")
            for q in range(NG):
                t = g * NG + q
                r0 = t * 128
                pp = min(128, N - r0)
                dt_ = dpool.tile([128, N], F32, tag="dt")
                nc.sync.dma_start(dt_[:pp, :], dist[s, r0:r0 + pp, :])
                # in-place negate: nd = -d
                nc.scalar.activation(dt_[:pp, :], dt_[:pp, :], AF.Copy,
                                     bias=0.0, scale=-1.0)
                m8 = sel.tile([128, 8], F32, tag="m8")
                nc.vector.max(m8[:pp, :], dt_[:pp, :])
                i16t = sel.tile([128, 8], U16, tag="i16")
                nc.vector.max_index(i16t[:pp, :], m8[:pp, :], dt_[:pp, :])
                # softmax over top-5 (shift-free: values in [-1, 0])
                e5 = sel.tile([128, 5], F32, tag="e5")
                z5 = sel.tile([128, 1], F32, tag="z5")
                nc.scalar.activation(e5[:pp, :], m8[:pp, 0:5], AF.Exp,
                                     accum_out=z5[:pp, :])
                r5 = sel.tile([128, 1], F32, tag="r5")
                nc.vector.reciprocal(r5[:pp, :], z5[:pp, :])
                w6 = sel.tile([128, 6], F16, tag="w6")
                nc.vector.memset(w6[:pp, 5:6], 0.0)
                nc.vector.tensor_scalar(w6[:pp, 0:5], e5[:pp, :], r5[:pp, :],
                                        None, OP.mult)
                # zero the m-padding; scatter never writes it
                nc.vector.memset(wn[:, q, N:NP], 0.0)
                if pp < 128:
                    # zero rows the scatter won't touch (pad nodes)
                    base = pp // 32 * 32
                    nc.vector.memset(wn[base:128, q, :], 0.0)
                # slot 5 carries the 6th-best index with weight 0.0
                nc.gpsimd.local_scatter(
                    wn[0:pp, q, 0:N], w6[:pp, :],
                    i16t[:pp, 0:6].bitcast(I16),
                    channels=(pp + 15) // 16 * 16, num_elems=N, num_idxs=6)

            if gather:
                emit_gather(st, g, wn)
            return wn

        def emit_agg_chunk(st, l, ch, aggT):
            ps = ps_agg.tile([128, CHUNK], F32, tag="ps_aggT")
            for c in range(NT):
                nc.tensor.matmul(ps[:, :], lhsT=st.h16[:, c, :],
                                 rhs=st.wt[ch][:, c, :],
                                 start=(c == 0), stop=(c == NT - 1))
            nc.any.tensor_copy(aggT[:, ch * CHUNK:(ch + 1) * CHUNK], ps[:, :])

        def emit_layer_rest(s, l, st, aggT):
            # deltaT = relu(W @ aggT + b); f32 (PE transpose needs
            # out dtype == lhsT dtype and r accumulates in f32)
            deltaT = agp.tile([128, NP], F32, tag="deltaT", bufs=1)
            for ch in range(NGRP):
                sl = slice(ch * CHUNK, (ch + 1) * CHUNK)
                psd = ps_d.tile([128, CHUNK], F32, tag="psd")
                nc.tensor.matmul(psd[:, :], lhsT=wsT_sb[:, l * H:(l + 1) * H],
                                 rhs=aggT[:, sl], start=True, stop=True)
                nc.scalar.activation(deltaT[:, sl], psd[:, :], AF.Relu,
                                     bias=bs_sb[:, l:l + 1], scale=1.0)
            # r = delta^T^T + h  (transpose + identity-matmul residual);
            # PSUM flags grouped per 2KB bank.
            r = ps_r.tile([128, NP], F32, tag="r")
            SPB = 512 // 128
            for g in range(NT // SPB):
                ts_ = list(range(g * SPB, (g + 1) * SPB))
                for j, t in enumerate(ts_):
                    sl = slice(t * 128, (t + 1) * 128)
                    nc.tensor.matmul(r[:, sl], lhsT=deltaT[:, sl],
                                     rhs=identf_sb[:, :], is_transpose=True,
                                     start=(j == 0), stop=False)
                for j, t in enumerate(ts_):
                    sl = slice(t * 128, (t + 1) * 128)
                    nc.tensor.matmul(r[:, sl], lhsT=ident_sb[:, :],
                                     rhs=st.h16[:, t, :], start=False,
                                     stop=(j == len(ts_) - 1))
            # layernorm stats (sq in fp16: |x|<~30, rel 5e-4 fine for var)
            sq = agp.tile([128, NP], F16, tag="sq", bufs=1)
            nc.scalar.square(sq[:, :], r[:, :])
            sr = ln.tile([128, NT], F32, tag="sr")
            nc.vector.tensor_reduce(
                sr[:, :], r[:, :].rearrange("p (t h) -> p t h", h=H),
                axis=mybir.AxisListType.X, op=OP.add)
            ssq = ln.tile([128, NT], F32, tag="ssq")
            nc.vector.tensor_reduce(
                ssq[:, :], sq[:, :].rearrange("p (t h) -> p t h", h=H),
                axis=mybir.AxisListType.X, op=OP.add)
            mu = ln.tile([128, NT], F32, tag="mu")
            nc.vector.tensor_scalar_mul(mu[:, :], sr[:, :], 1.0 / H)
            var = ln.tile([128, NT], F32, tag="var")
            # var = E[x^2] - mu^2  (+eps folded into sqrt bias)
            nc.vector.tensor_scalar_mul(var[:, :], ssq[:, :], 1.0 / H)
            musq = ln.tile([128, NT], F32, tag="musq")
            nc.vector.tensor_tensor(musq[:, :], mu[:, :], mu[:, :], OP.mult)
            nc.vector.tensor_tensor(var[:, :], var[:, :], musq[:, :], OP.subtract)
            # sinv = (var+eps)^-0.5 via exp(-0.5*ln(var+eps)): keeps ACT on
            # one table set (exp/ln) -- no sqrt-set reload on the LN path.
            lnv = ln.tile([128, NT], F32, tag="lnv")
            nc.scalar.activation(lnv[:, :], var[:, :], AF.Ln,
                                 bias=eps_sb[:, :], scale=1.0)
            sinv = ln.tile([128, NT], F32, tag="sinv")
            nc.scalar.activation(sinv[:, :], lnv[:, :], AF.Exp,
                                 bias=0.0, scale=-0.5)
            negmus = ln.tile([128, NT], F32, tag="negmus")
            nc.vector.tensor_tensor(negmus[:, :], mu[:, :], sinv[:, :], OP.mult)
            nc.vector.tensor_scalar(negmus[:, :], negmus[:, :], -1.0, None,
                                    OP.mult)
            # normalize: h = (r - mu) * sinv  [* gamma + beta]
            last = l == L - 1
            if last:
                st.hout = hop.tile([128, NT, H], F32, tag="hout")
                dst = st.hout
            else:
                dst = st.h16
            for t in range(NT):
                sl = slice(t * 128, (t + 1) * 128)
                if trivial_affine:
                    if t % 4 == 3:
                        nc.vector.tensor_scalar(dst[:, t, :], r[:, sl],
                                                sinv[:, t:t + 1],
                                                negmus[:, t:t + 1],
                                                OP.mult, OP.add)
                    else:
                        nc.scalar.activation(dst[:, t, :], r[:, sl],
                                             AF.Identity,
                                             bias=negmus[:, t:t + 1],
                                             scale=sinv[:, t:t + 1])
                else:
                    tmp = ln.tile([128, H], F32, tag="nrmtmp")
                    nc.scalar.activation(tmp[:, :], r[:, sl], AF.Identity,
                                         bias=negmus[:, t:t + 1],
                                         scale=sinv[:, t:t + 1])
                    nc.vector.tensor_tensor(
                        tmp[:, :], tmp[:, :],
                        grep_sb[:, l * H:(l + 1) * H], OP.mult)
                    nc.vector.tensor_tensor(
                        dst[:, t, :], tmp[:, :],
                        brep_sb[:, l * H:(l + 1) * H], OP.add)

        def emit_layer(s, l, st):
            aggT = agp.tile([128, NP], F16, tag="aggT", bufs=3)
            for ch in range(NGRP):
                emit_agg_chunk(st, l, ch, aggT)
            emit_layer_rest(s, l, st, aggT)

        def emit_store(s, st):
            nc.sync.dma_start(
                out[s, 0:(NT - 1) * 128, :].rearrange("(t p) h -> p t h", p=128),
                st.hout[:, 0:NT - 1, :])
            nc.sync.dma_start(out[s, (NT - 1) * 128:N, :],
                              st.hout[0:N - (NT - 1) * 128, NT - 1, :])

        # ---- software-pipelined emission over the SPC samples ----------
        # s0: sel+L0agg interleaved, L0 rest, L1 (s1's selection scatters
        # overlap on DVE/ACT/GpSimd), then s0's L2 agg chunks back-to-back
        # (the L2 agg chunk g is the last reader of wt{g}), all four s1
        # gathers concurrently, L2 rest + store overlapping them; s1: L0,
        # L1, L2, store.
        s0 = _SampleState()
        s1 = _SampleState()
        emit_emb(0, s0)
        emit_emb(1, s1)
        a00 = agp.tile([128, NP], F16, tag="aggT", name="aggT_s0l0", bufs=3)
        for g in range(NGRP):
            emit_sel_group(0, g, s0)
            emit_agg_chunk(s0, 0, g, a00)
        emit_layer_rest(0, 0, s0, a00)
        wn1 = [emit_sel_group(1, g, s1, gather=False) for g in range(2)]
        emit_layer(0, 1, s0)
        wn1 += [emit_sel_group(1, g, s1, gather=False) for g in range(2, NGRP)]
        a02 = agp.tile([128, NP], F16, tag="aggT", name="aggT_s0l2", bufs=3)
        for g in range(NGRP):
            emit_agg_chunk(s0, 2, g, a02)
        for g in range(NGRP):
            emit_gather(s1, g, wn1[g])
        emit_layer_rest(0, 2, s0, a02)
        emit_store(0, s0)
        a10 = agp.tile([128, NP], F16, tag="aggT", name="aggT_s1l0", bufs=3)
        for g in range(NGRP):
            emit_agg_chunk(s1, 0, g, a10)
        emit_layer_rest(1, 0, s1, a10)
        emit_layer(1, 1, s1)
        emit_layer(1, 2, s1)
        emit_store(1, s1)

    if compile:
        nc.compile()
    return nc


def _gather_idx_table():
    """Local index i of the gather lives at (partition i%16 + 16*core,
    free i//16), replicated across the 8 gpsimd cores; value = rank-local
    row (rank = i//128 selects the group tile's NG stripes)."""
    vals = np.arange(CHUNK, dtype=np.int16)
    tab = np.zeros((128, CHUNK // 16), dtype=np.int16)
    for g in range(8):
        tab[g * 16:(g + 1) * 16, :] = vals.reshape(CHUNK // 16, 16).T
    return tab


_CACHE = {}


def kernel(node_emb, dist_matrix, Ws, bs, gammas, betas):
    node_emb = np.ascontiguousarray(np.asarray(node_emb, dtype=np.float32))
    dist_matrix = np.ascontiguousarray(np.asarray(dist_matrix, dtype=np.float32))
    Ws = np.asarray(Ws, dtype=np.float32)
    bs = np.asarray(bs, dtype=np.float32)
    gammas = np.asarray(gammas, dtype=np.float32)
    betas = np.asarray(betas, dtype=np.float32)

    trivial = bool(np.all(gammas == 1.0) and np.all(betas == 0.0))
    key = ("prog", trivial)
    if key not in _CACHE:
        _CACHE[key] = _build_program(trivial)
    nc = _CACHE[key]

    wsT = np.ascontiguousarray(np.transpose(Ws, (0, 2, 1)).astype(np.float16))
    bs3 = np.ascontiguousarray(bs[:, :, None])
    gtab = _gather_idx_table()

    in_maps = []
    for c in range(N_CORES):
        m = {
            "dist": dist_matrix[c * SPC:(c + 1) * SPC],
            "emb": node_emb[c * SPC:(c + 1) * SPC],
            "wsT": wsT,
            "bs": bs3,
            "ident": np.eye(H, dtype=np.float16),
            "identf": np.eye(H, dtype=np.float32),
            "gidx": gtab,
        }
        if not trivial:
            m["grep"] = np.ascontiguousarray(
                np.broadcast_to(gammas[:, None, :], (L, H, H)))
            m["brep"] = np.ascontiguousarray(
                np.broadcast_to(betas[:, None, :], (L, H, H)))
        in_maps.append(m)

    res = run_bass_kernel_spmd(nc, in_maps, list(range(N_CORES)))
    kernel.last_results = res
    out = np.concatenate([res.results[c]["out"] for c in range(N_CORES)], axis=0)
    return out
